# revision 2
# baseline (speedup 1.0000x reference)
"""Trainium2 Bass kernel for nn_KANLayer (B=16384, D=1024, K=8).

Math: the per-feature basis chain collapses algebraically:
    nl[b,i] = sum_k (x[b,i]*W1[i,k] + b1[i,k]) * W2[i,k]
            = x[b,i] * a[i] + c[i],   a = sum_k W1*W2, c = sum_k b1*W2
so the whole layer is ONE dense matmul with a fused diagonal + bias:
    out = x @ (lin_W.T + diag(a)) + (lin_b + c)

Sharding: data-parallel over batch across 8 NeuronCores (2048 rows each);
W_eff (1024x1024) + bias replicated. No collectives needed.

Device kernel (per core): out[2048,1024] = xT.T @ W_eff + bias
  - lhsT (stationary) = xT tile [128 j, 128 b]  (x transposed on host)
  - rhs  (moving)     = W_eff [128 j, 512 i] slices, resident in SBUF
  - psum [128 b, 512 i] f32, accumulated over 8 k-subtiles
  - bias added during PSUM->SBUF eviction on the vector engine
"""

import os
from contextlib import ExitStack

import numpy as np
import ml_dtypes

import concourse.bass as bass
import concourse.tile as tile
from concourse import bacc, mybir
from concourse.bass_utils import run_bass_kernel_spmd

B, D = 16384, 1024
NCORES = 8
BS = B // NCORES  # 2048 batch rows per core
P = 128
KT = D // P   # 8 contraction subtiles
NB = BS // P  # 16 batch tiles per core
NCH = D // 512  # 2 output-feature chunks of 512

# matmul input dtype: bf16 (1 cyc/row on PE, f32 PSUM accumulate)
MM_DT = mybir.dt.bfloat16
MM_NP = ml_dtypes.bfloat16

_CACHE = {}


def _build_nc():
    nc = bacc.Bacc("TRN2", target_bir_lowering=False, debug=False,
                   num_devices=NCORES)
    xT = nc.dram_tensor("xT", [D, BS], MM_DT, kind="ExternalInput").ap()
    w = nc.dram_tensor("w", [D, D], MM_DT, kind="ExternalInput").ap()
    bias = nc.dram_tensor("bias", [D], mybir.dt.float32,
                          kind="ExternalInput").ap()
    out = nc.dram_tensor("out", [BS, D], mybir.dt.float32,
                         kind="ExternalOutput").ap()

    # DRAM views
    # xT[j, b] -> [nb, p(j-sub), kt, b] so one DMA per batch tile grabs all
    # 8 k-subtile blocks [128j x 128b] for that tile.
    xT_r = xT.rearrange("(kt p) (nb b) -> nb p kt b", p=P, b=P)
    w_r = w.rearrange("(kt p) n -> kt p n", p=P)
    out_r = out.rearrange("(nb p) n -> nb p n", p=P)

    with tile.TileContext(nc) as tc, ExitStack() as ctx:
        wpool = ctx.enter_context(tc.tile_pool(name="wpool", bufs=1))
        xpool = ctx.enter_context(tc.tile_pool(name="xpool", bufs=4))
        opool = ctx.enter_context(tc.tile_pool(name="opool", bufs=3))
        ppool = ctx.enter_context(tc.tile_pool(name="ppool", bufs=4,
                                               space="PSUM"))

        # Resident weights: 8 tiles [128, 1024], one per k-subtile
        w_t = []
        for kt in range(KT):
            wt = wpool.tile([P, D], MM_DT, tag=f"w{kt}", name=f"w_t{kt}")
            nc.sync.dma_start(out=wt, in_=w_r[kt])
            w_t.append(wt)

        # Bias broadcast across partitions: [128, 1024] f32
        bias_t = wpool.tile([P, D], mybir.dt.float32, tag="bias",
                            name="bias_t")
        bias_bc = bass.AP(tensor=bias.tensor, offset=bias.offset,
                          ap=[[0, P], bias.ap[0]])
        nc.gpsimd.dma_start(out=bias_t, in_=bias_bc)

        for bt in range(NB):
            x_t = xpool.tile([P, KT, P], MM_DT, tag="x", name=f"x_t{bt}")
            nc.sync.dma_start(out=x_t, in_=xT_r[bt])
            o_t = opool.tile([P, D], mybir.dt.float32, tag="o",
                             name=f"o_t{bt}")
            for ch in range(NCH):
                psum = ppool.tile([P, 512], mybir.dt.float32, tag="ps",
                                  name=f"ps{bt}_{ch}")
                for kt in range(KT):
                    nc.tensor.matmul(
                        psum,
                        lhsT=x_t[:, kt, :],
                        rhs=w_t[kt][:, bass.ts(ch, 512)],
                        start=(kt == 0),
                        stop=(kt == KT - 1),
                    )
                nc.vector.tensor_add(o_t[:, bass.ts(ch, 512)], psum,
                                     bias_t[:, bass.ts(ch, 512)])
            nc.sync.dma_start(out=out_r[bt], in_=o_t)

    nc.compile()
    return nc


def _get_nc():
    if "nc" not in _CACHE:
        _CACHE["nc"] = _build_nc()
    return _CACHE["nc"]


def _prep_inputs(x, lin_W, lin_b, W1, b1, W2):
    """Host-side prep: fold the per-feature basis chain into the matmul."""
    x = np.asarray(x, dtype=np.float32)
    lin_W = np.asarray(lin_W, dtype=np.float32)
    a = np.sum(np.asarray(W1, np.float32) * np.asarray(W2, np.float32),
               axis=1)
    c = np.sum(np.asarray(b1, np.float32) * np.asarray(W2, np.float32),
               axis=1)
    W_eff = np.ascontiguousarray(lin_W.T)
    idx = np.arange(D)
    W_eff[idx, idx] += a
    bias = (np.asarray(lin_b, np.float32) + c).astype(np.float32)

    xT = np.ascontiguousarray(x.T).astype(MM_NP)  # [D, B]
    w_dev = W_eff.astype(MM_NP)
    return xT, w_dev, bias


def kernel(x, lin_W, lin_b, W1, b1, W2):
    xT, w_dev, bias = _prep_inputs(x, lin_W, lin_b, W1, b1, W2)
    in_maps = [
        {
            "xT": np.ascontiguousarray(xT[:, i * BS:(i + 1) * BS]),
            "w": w_dev,
            "bias": bias,
        }
        for i in range(NCORES)
    ]
    nc = _get_nc()
    res = run_bass_kernel_spmd(nc, in_maps, core_ids=list(range(NCORES)))
    out = np.concatenate([r["out"] for r in res.results], axis=0)
    return np.ascontiguousarray(out.astype(np.float32))
